# revision 20
# baseline (speedup 1.0000x reference)
"""Trainium2 Bass kernel for nn_EventMemoryCell (B=4096, D=H=512, S=16).

Strategy (hardcoded for the spec shapes):
  - Data parallel over batch across 8 cores (512 rows each), parameters
    replicated; one SPMD NEFF.
  - Everything on-device runs in a transposed (feature-on-partition,
    batch-on-free) layout, so every matmul contracts over partitions and
    the LSTM recurrence needs no transposes.
  - mem_seq is never materialized: for s<15,
      xg[s] = A@slots_old[s+1] + C@cum_old[s+1] + d*(delta_old[s+1]+1)
              + bias + shared,  shared = A@(leak*x) + 2*C@x
    and for s=15 xg[15] = A@new_slot + bias + shared.
  - Attention: sims = slots . ((Wk^T Wq) x), so keys (B,S,H) is never built.
  - dtypes: the A/C stream matmuls, the W_hh recurrence, the sims reduce
    and the per-step rank-2 (bias + d*delta) term all run in fp8(e4m3)
    with MatmulPerfMode.DoubleRow (K=256 per instruction, 2x PE
    throughput). fp8-path weights are pre-scaled by SC=64 host-side and
    the factor is divided back out via the activation `scale` operand.
    `shared` is computed once in fp16 (its error would be coherently
    amplified ~16x across steps), and the output projection is fp16.
    Cell state, gates and h are fp16; h is written as fp8 for the
    recurrence matmul (fp16 only at the last step, for the projection).
  - The per-step rank-2 term rides the matmul group as a DoubleRow
    matmul whose stationary has [bias_row; d_row] on partition 0 and
    zeros elsewhere (full-width loads keep dual-fp8 LDWEIGHTS fast),
    so no vector-engine STT is needed.
  - Scheduling: each step opens its first 6 gate-tile groups with only
    h-independent terms so the PE stays busy while the previous step's
    h tail (act -> c-chain -> tanh -> mul) drains; the sims reduce and
    the new-slot block are emitted mid-step for the same reason. The
    cell-state chain runs on the Pool engine (DVE does the PSUM adds).
"""
import sys

sys.path.insert(0, "/opt/trn_rl_repo")

import numpy as np
import ml_dtypes

import concourse.bass as bass
import concourse.tile as tile
import concourse.mybir as mybir
from concourse import bacc, bass_utils

F32 = mybir.dt.float32
F32R = mybir.dt.float32r
F16 = mybir.dt.float16
F8 = mybir.dt.float8e4
AF = mybir.ActivationFunctionType
DR = mybir.MatmulPerfMode.DoubleRow
NP8 = ml_dtypes.float8_e4m3

B, S, D, H = 4096, 16, 512, 512
NCORES = 8
BC = B // NCORES            # 512 batch rows per core
G4 = 4 * H                  # 2048 gate rows
KD = D // 128               # 4 k-tiles over D/H
KM = G4 // 128              # 16 gate partition tiles
KO = (2 * H + D) // 128     # 12 k-tiles for the output projection
SC = 64.0                   # fp8 weight pre-scale
SCI = 1.0 / SC
NPRE = 6                    # h-independent prefix groups per step

# interleaved gate-tile order [0,4,8,12, 1,5,9,13, ...]: finish chunk j's
# i/f/g/o gates together so c/h updates start early
M_ORDER = [j + 4 * i for j in range(4) for i in range(4)]

_BUILT = None
DEBUG_STEPS = ()  # set before first kernel() call to dump c/h after these steps


def _build_program():
    nc = bacc.Bacc("TRN2", target_bir_lowering=False, debug=False)

    def din(name, shape, dt):
        return nc.dram_tensor(name, list(shape), dt, kind="ExternalInput").ap()

    xT = din("xT", (D, BC), F32R)
    x16T = din("x16T", (D, BC), F16)
    x216T = din("x216T", (D, BC), F16)
    s0T = din("s0T", (D, BC), F16)
    slotsT = din("slotsT", (S - 1, D, BC), F8)
    cumT = din("cumT", (S - 1, D, BC), F8)
    d1T = din("d1T", (1, S * BC), F16)
    hpT = din("hpT", (H, BC), F16)
    m2T = din("m2T", (D, D), F32R)
    wvT = din("wvT", (D, D), F32R)
    wrT = din("wrT", (D, 1), F32R)
    waT = din("waT", (D, 1), F32R)
    bv = din("bv", (D, 1), F32)
    br = din("br", (1, 1), F32)
    ba = din("ba", (1, 1), F32)
    bo = din("bo", (H, 1), F32)
    aT = din("aT", (D, G4), F8)
    cT = din("cT", (D, G4), F8)
    aT16 = din("aT16", (D, G4), F16)
    cT16 = din("cT16", (D, G4), F16)
    bd8 = din("bd8", (128, 2 * G4), F8)
    whT = din("whT", (H, G4), F8)
    woT = din("woT", (2 * H + D, H), F16)
    hnT = nc.dram_tensor("hnT", [H, BC], F32, kind="ExternalOutput").ap()
    dbg = {}
    for ds in DEBUG_STEPS:
        dbg[ds] = (nc.dram_tensor(f"cD{ds}", [H, BC], F32, kind="ExternalOutput").ap(),
                   nc.dram_tensor(f"hD{ds}", [H, BC], F32, kind="ExternalOutput").ap())

    r3 = lambda ap: ap.rearrange("(kt p) b -> p kt b", p=128)
    r2 = lambda ap: ap.rearrange("(kt p) o -> p (kt o)", p=128)

    with tile.TileContext(nc) as tc:
        wp = tc.alloc_tile_pool(name="wp", bufs=1)
        st_p = tc.alloc_tile_pool(name="state", bufs=1)
        pp = tc.alloc_tile_pool(name="pp", bufs=8, space="PSUM")
        sp = tc.alloc_tile_pool(name="sp", bufs=2)
        cp = tc.alloc_tile_pool(name="cp", bufs=2)
        gp = tc.alloc_tile_pool(name="gp", bufs=6)
        tp = tc.alloc_tile_pool(name="tp", bufs=2)

        # ---- resident tiles (DMAs emitted later, in priority order) ----
        aT_sb = wp.tile([128, KD, G4], F8, name="aT_sb")
        cT_sb = wp.tile([128, KD, G4], F8, name="cT_sb")
        whT_sb = wp.tile([128, KD, G4], F8, name="whT_sb")
        # rank-2 (bias + d*delta) stationary: only partition 0 is nonzero
        # ([bias_row; d_row]); zero rows elsewhere keep the LDWEIGHTS a
        # full-width (fast) dual-fp8 load while contributing nothing.
        bd_sb = wp.tile([128, 2, G4], F8, name="bd_sb")
        d1_sb = wp.tile([1, S * BC], F16, name="d1_sb")
        # moving rows for the rank-2 term: partition 0 = [ones; delta_s],
        # partitions 1..127 zeroed once (they hit zero weights; zeroing
        # avoids NaN*0 from uninitialized fp8 bytes). Double-buffered so
        # the per-step delta write never races the previous step's reads.
        dl_bc = []
        for pq in range(2):
            t = wp.tile([128, 2, BC], F8, name=f"dl_bc{pq}")
            nc.vector.memset(t[:], 0.0)
            nc.vector.memset(t[0:1, 0, :], 1.0)
            dl_bc.append(t)
        wo_sb = wp.tile([128, KO, H], F16, name="wo_sb")
        bo_sb = wp.tile([128, KD], F32, name="bo_sb")
        ones_bf = wp.tile([1, BC], F16, name="ones_bf")
        nc.vector.memset(ones_bf[:], 1.0)
        ones8 = wp.tile([128, 2, 128], F8, name="ones8")
        nc.vector.memset(ones8[:], 1.0)

        xt = st_p.tile([128, KD, BC], F32R, name="xt")
        x16 = st_p.tile([128, KD, BC], F16, name="x16")
        hp16 = st_p.tile([128, KD, BC], F16, name="hp16")
        shared_sb = st_p.tile([128, KM, BC], F16, name="shared_sb")
        ut8 = st_p.tile([128, KD, BC], F8, name="ut8")
        P_t = st_p.tile([128, KD, BC], F16, name="P_t")
        c_t = [st_p.tile([128, BC], F16, name=f"c{k}", tag=f"c{k}") for k in range(KD)]
        h8_t = [st_p.tile([128, KD, BC], F8, name=f"h8_{pq}") for pq in range(2)]
        h15 = st_p.tile([128, KD, BC], F16, name="h15")
        g_row = st_p.tile([1, BC], F16, name="g_row")
        max_row = st_p.tile([1, BC], F32, name="max_row")

        # step-0 stream tiles, hoisted so their DMAs can be prioritized
        st0 = sp.tile([128, KD, BC], F8, name="st0", tag="st")
        ct0 = cp.tile([128, KD, BC], F8, name="ct0", tag="ct")

        MSIG, MTANH = AF.Sigmoid, AF.Tanh

        def mm_group2(ps_ap, terms, start, stop):
            n = len(terms)
            for i, t in enumerate(terms):
                lh, rh = t[0], t[1]
                pm = t[2] if len(t) > 2 else None
                nc.tensor.matmul(ps_ap, lh, rh, start=(start and i == 0),
                                 stop=(stop and i == n - 1), perf_mode=pm)

        def mm_group(ps_ap, terms):
            mm_group2(ps_ap, terms, True, True)

        def dr_terms(w_sb, rhs_sb, sl):
            # K=512 contraction as 2 DoubleRow fp8 matmuls
            return [(w_sb[:, 2 * i:2 * i + 2, sl], rhs_sb[:, 2 * i:2 * i + 2, :], DR)
                    for i in range(2)]

        def sims_row(idx, ts8, tagsfx, tpool):
            # running max over slots: max_row = max(max_row, slots_s . u)
            # all-ones stationary gives 128 identical sum rows (dual-fp8
            # LDWEIGHTS rejects a 1-wide stationary); row 0 is read out
            srp = pp.tile([128, BC], F32, name=f"srp{tagsfx}", tag="ps")
            mm_group(srp[:], [(ones8[:], ts8[:, 2 * i:2 * i + 2, :], DR)
                              for i in range(2)])
            if idx == 0:
                nc.scalar.activation(max_row[:], srp[0:1, :], AF.Copy)
            else:
                srow = tpool.tile([1, BC], F32, name=f"sr{tagsfx}", tag="srow",
                                  bufs=1)
                nc.scalar.activation(srow[:], srp[0:1, :], AF.Copy)
                nc.vector.tensor_max(max_row[:], max_row[:], srow[:])

        # ================= prologue =================
        with tc.tile_pool(name="prop", bufs=1) as prop:
            s0_sb = prop.tile([128, KD, BC], F16, name="s0_sb")
            x216 = prop.tile([128, KD, BC], F16, name="x216")
            vt = prop.tile([128, KD, BC], F16, name="vt")
            lx16 = prop.tile([128, KD, BC], F16, name="lx16")
            r_row = prop.tile([1, BC], F16, name="r_row")
            lk_row = prop.tile([1, BC], F16, name="lk_row")
            R_bc = prop.tile([128, BC], F16, name="R_bc")
            L_bc = prop.tile([128, BC], F16, name="L_bc")
            ts0 = prop.tile([128, KD, BC], F8, name="ts0")

            with tc.tile_pool(name="mmp", bufs=1) as mmp:
                m2_sb = mmp.tile([128, KD, D], F32R, name="m2_sb")
                wv_sb = mmp.tile([128, KD, D], F32R, name="wv_sb")
                wr_sb = mmp.tile([128, KD], F32R, name="wr_sb")
                wa_sb = mmp.tile([128, KD], F32R, name="wa_sb")
                bv_sb = mmp.tile([128, KD], F32, name="bv_sb")
                br_sb = mmp.tile([1, 1], F32, name="br_sb")
                ba_sb = mmp.tile([1, 1], F32, name="ba_sb")

                # DMAs in consumption order: the PE hits u first, then
                # r/leak/v, then `shared` (second scope), then step 0's
                # stream groups; the recurrence weights one step later.
                nc.sync.dma_start(xt[:], r3(xT))
                nc.sync.dma_start(m2_sb[:], r3(m2T))
                nc.sync.dma_start(wr_sb[:], r2(wrT))
                nc.sync.dma_start(wa_sb[:], r2(waT))
                nc.sync.dma_start(bv_sb[:], r2(bv))
                nc.sync.dma_start(br_sb[:], br)
                nc.sync.dma_start(ba_sb[:], ba)
                nc.sync.dma_start(d1_sb[:], d1T)
                nc.sync.dma_start(bo_sb[:], r2(bo))
                nc.sync.dma_start(x16[:], r3(x16T))
                nc.sync.dma_start(s0_sb[:], r3(s0T))
                nc.sync.dma_start(wv_sb[:], r3(wvT))
                nc.sync.dma_start(x216[:], r3(x216T))

                # u = (Wk^T Wq) x   (feature-major)
                for m in range(KD):
                    ups = pp.tile([128, BC], F32, name=f"ups{m}", tag="ps")
                    mm_group(ups[:], [(m2_sb[:, k, 128 * m:128 * (m + 1)],
                                       xt[:, k, :]) for k in range(KD)])
                    nc.scalar.activation(ut8[:, m, :], ups[:], AF.Copy)

                # r / leak rows
                rps = pp.tile([128, BC], F32, name="rps", tag="ps")
                mm_group(rps[0:1, :], [(wr_sb[:, k:k + 1], xt[:, k, :])
                                       for k in range(KD)])
                nc.scalar.activation(r_row[:], rps[0:1, :], MSIG,
                                     bias=br_sb[0:1, 0:1])
                lps = pp.tile([128, BC], F32, name="lps", tag="ps")
                mm_group(lps[0:1, :], [(wa_sb[:, k:k + 1], xt[:, k, :])
                                       for k in range(KD)])
                nc.scalar.activation(lk_row[:], lps[0:1, :], MSIG,
                                     bias=ba_sb[0:1, 0:1])

                # broadcast r/leak rows to 128 partitions via a K=1 matmul
                bps = pp.tile([128, BC], F32, name="bps", tag="ps")
                mm_group(bps[:], [(ones_bf[0:1, 0:128], r_row[:])])
                nc.scalar.activation(R_bc[:], bps[:], AF.Copy)
                bps2 = pp.tile([128, BC], F32, name="bps2", tag="ps")
                mm_group(bps2[:], [(ones_bf[0:1, 0:128], lk_row[:])])
                nc.scalar.activation(L_bc[:], bps2[:], AF.Copy)

                # lx = leak*x ; sims row 0 (original slot 0)
                for k in range(KD):
                    nc.vector.tensor_mul(lx16[:, k, :], L_bc[:], x16[:, k, :])
                nc.vector.tensor_mul(ts0[:], s0_sb[:], ut8[:])
                sims_row(0, ts0, "p", prop)

                # v = Wv x + bv ; P = r*slots0 + (1-r)*v
                for m in range(KD):
                    vps = pp.tile([128, BC], F32, name=f"vps{m}", tag="ps")
                    mm_group(vps[:], [(wv_sb[:, k, 128 * m:128 * (m + 1)],
                                       xt[:, k, :]) for k in range(KD)])
                    nc.scalar.activation(vt[:, m, :], vps[:], AF.Identity,
                                         bias=bv_sb[:, m:m + 1])
                for k in range(KD):
                    t1 = prop.tile([128, BC], F16, name=f"pt{k}", tag="ptmp",
                                   bufs=2)
                    nc.vector.tensor_sub(t1[:], s0_sb[:, k, :], vt[:, k, :])
                    nc.vector.tensor_mul(t1[:], R_bc[:], t1[:])
                    nc.vector.tensor_add(P_t[:, k, :], vt[:, k, :], t1[:])

            with tc.tile_pool(name="acp", bufs=1) as acp:
                a16_sb = acp.tile([128, KD, G4], F16, name="a16_sb")
                c16_sb = acp.tile([128, KD, G4], F16, name="c16_sb")
                nc.sync.dma_start(a16_sb[:], r3(aT16))
                nc.sync.dma_start(c16_sb[:], r3(cT16))
                nc.sync.dma_start(st0[:], r3(slotsT[0]))
                nc.sync.dma_start(ct0[:], r3(cumT[0]))
                nc.sync.dma_start(aT_sb[:], r3(aT))
                nc.sync.dma_start(cT_sb[:], r3(cT))
                nc.sync.dma_start(whT_sb[:], r3(whT))
                nc.sync.dma_start(bd_sb[:], bd8.rearrange("p (t g) -> p t g", t=2))
                nc.sync.dma_start(hp16[:], r3(hpT))

                # shared = A@lx + C@(2x)  in fp16 (scaled by SC; no bias --
                # bias rides the per-step rank-2 bd term)
                for m in range(KM):
                    sl = slice(128 * m, 128 * (m + 1))
                    sps = pp.tile([128, BC], F32, name=f"sps{m}", tag="ps")
                    terms = [(a16_sb[:, k, sl], lx16[:, k, :]) for k in range(KD)]
                    terms += [(c16_sb[:, k, sl], x216[:, k, :]) for k in range(KD)]
                    mm_group(sps[:], terms)
                    nc.scalar.activation(shared_sb[:, m, :], sps[:], AF.Copy)

            # stage step 0's delta row into the rank-2 moving tile
            nc.scalar.activation(dl_bc[0][0:1, 1, :], d1_sb[0:1, 0:BC], AF.Copy)

        # ================= LSTM over S steps =================
        ns8 = None
        for s in range(S):
            last = s == S - 1
            h8_rd = h8_t[(s + 1) % 2]  # h[s-1] (fp8)
            h8_wr = h8_t[s % 2]        # h[s]
            db = dl_bc[s % 2]
            if s == 0:
                st, ct = st0, ct0
            elif not last:
                st = sp.tile([128, KD, BC], F8, name=f"st{s}", tag="st")
                nc.sync.dma_start(st[:], r3(slotsT[s]))
                ct = cp.tile([128, KD, BC], F8, name=f"ct{s}", tag="ct")
                nc.sync.dma_start(ct[:], r3(cumT[s]))
            if not last:
                # stage the NEXT step's delta row (runs inside this step's
                # scalar stream, well before step s+1's matmuls need it)
                nc.scalar.activation(dl_bc[(s + 1) % 2][0:1, 1, :],
                                     d1_sb[0:1, (s + 1) * BC:(s + 2) * BC],
                                     AF.Copy)
            if s == 10:
                nc.sync.dma_start(wo_sb[:], r3(woT))
            tsim = None
            if not last:
                tsim = tp.tile([128, KD, BC], F8, name=f"tm{s}", tag="tsim",
                               bufs=2)
                nc.gpsimd.tensor_mul(tsim[:], st[:], ut8[:])

            def acd_terms(sl):
                if last:
                    t = dr_terms(aT_sb, ns8, sl)
                else:
                    t = dr_terms(aT_sb, st, sl) + dr_terms(cT_sb, ct, sl)
                # rank-2 (bias + d*delta_s) term
                return t + [(bd_sb[:, :, sl], db[:], DR)]

            # h-independent prefix: open the first NPRE m-groups with their
            # A/C/rank-2 terms so the PE stays busy while the previous
            # step's h8 tail (act -> c-chain -> tanh -> mul) drains.
            gates = [None] * KM
            ps_pre = {}
            for m in M_ORDER[:NPRE]:
                sl = slice(128 * m, 128 * (m + 1))
                ps = pp.tile([128, BC], F32, name=f"ps_{s}_{m}", tag="ps")
                ps_pre[m] = ps
                mm_group2(ps[:], acd_terms(sl), start=True, stop=(s == 0))
            for pos, m in enumerate(M_ORDER):
                if pos == 8 and not last:
                    # sims row for original slot s+1 (emitted mid-step so the
                    # tsim product has certainly landed)
                    sims_row(s + 1, tsim, str(s), tp)
                if pos == 12 and s == S - 2:
                    # g = sigmoid(max_s sims);  new_slot = g * P
                    nc.scalar.activation(g_row[:], max_row[:], MSIG)
                    gps = pp.tile([128, BC], F32, name="gps", tag="ps")
                    mm_group(gps[:], [(ones_bf[0:1, 0:128], g_row[:])])
                    G_bc = tp.tile([128, BC], F16, name="G_bc",
                                   tag="gbc", bufs=1)
                    nc.scalar.activation(G_bc[:], gps[:], AF.Copy)
                    ns8 = sp.tile([128, KD, BC], F8, name="ns8", tag="st")
                    for k in range(KD):
                        nc.vector.tensor_mul(ns8[:, k, :], G_bc[:], P_t[:, k, :])
                sl = slice(128 * m, 128 * (m + 1))
                if pos < NPRE:
                    ps = ps_pre[m]
                    if s > 0:
                        mm_group2(ps[:], dr_terms(whT_sb, h8_rd, sl),
                                  start=False, stop=True)
                else:
                    ps = pp.tile([128, BC], F32, name=f"ps_{s}_{m}", tag="ps")
                    terms = acd_terms(sl)
                    if s > 0:
                        terms += dr_terms(whT_sb, h8_rd, sl)
                    mm_group(ps[:], terms)
                nc.vector.tensor_add(ps[:], ps[:], shared_sb[:, m, :])
                gt = gp.tile([128, BC], F16, name=f"g_{s}_{m}", tag="gate")
                nc.scalar.activation(gt[:], ps[:], MTANH if m // 4 == 2 else MSIG,
                                     scale=SCI)
                gates[m] = gt
                # after chunk j completes (i,f,g,o present), update c/h:
                # t2 on DVE in parallel with the Pool-engine c ops
                j = m - 12
                if j >= 0:
                    ig, fg, gg, og = (gates[j], gates[4 + j], gates[8 + j],
                                      gates[12 + j])
                    tct = tp.tile([128, BC], F16, name=f"t_{s}_{j}", tag="tct")
                    if s == 0:
                        nc.gpsimd.tensor_mul(c_t[j][:], ig[:], gg[:])
                    else:
                        t2 = tp.tile([128, BC], F16, name=f"u_{s}_{j}", tag="t2")
                        nc.vector.tensor_mul(t2[:], fg[:], c_t[j][:])
                        nc.gpsimd.tensor_mul(c_t[j][:], ig[:], gg[:])
                        nc.gpsimd.tensor_add(c_t[j][:], c_t[j][:], t2[:])
                    nc.scalar.activation(tct[:], c_t[j][:], MTANH)
                    if last:
                        nc.vector.tensor_mul(h15[:, j, :], og[:], tct[:])
                    else:
                        nc.vector.tensor_mul(h8_wr[:, j, :], og[:], tct[:])
            if s in dbg:
                cD, hD = dbg[s]
                for j in range(KD):
                    ccp = tp.tile([128, BC], F32, name=f"ccp{s}_{j}", tag="hcp")
                    nc.scalar.activation(ccp[:], c_t[j][:], AF.Copy)
                    nc.sync.dma_start(cD[128 * j:128 * (j + 1), :], ccp[:])
                    hsrc = h15[:, j, :] if last else h8_wr[:, j, :]
                    hcp = tp.tile([128, BC], F32, name=f"hcp{s}_{j}", tag="hcp")
                    nc.scalar.activation(hcp[:], hsrc, AF.Copy)
                    nc.sync.dma_start(hD[128 * j:128 * (j + 1), :], hcp[:])

        # ================= epilogue =================
        # same split trick: the hp/x terms are h15-independent and cover
        # the last step's h tail
        with tc.tile_pool(name="ep", bufs=1) as ep:
            eps_t = []
            for m in range(KD):
                sl = slice(128 * m, 128 * (m + 1))
                eps = pp.tile([128, BC], F32, name=f"eps{m}", tag="ps")
                eps_t.append(eps)
                terms = [(wo_sb[:, 4 + j, sl], hp16[:, j, :]) for j in range(KD)]
                terms += [(wo_sb[:, 8 + j, sl], x16[:, j, :]) for j in range(KD)]
                mm_group2(eps[:], terms, start=True, stop=False)
            for m in range(KD):
                sl = slice(128 * m, 128 * (m + 1))
                mm_group2(eps_t[m][:], [(wo_sb[:, j, sl], h15[:, j, :])
                                        for j in range(KD)], start=False, stop=True)
                out_t = ep.tile([128, BC], F32, name=f"o{m}", tag="out", bufs=2)
                nc.scalar.activation(out_t[:], eps_t[m][:], MTANH,
                                     bias=bo_sb[:, m:m + 1])
                nc.sync.dma_start(hnT[128 * m:128 * (m + 1), :], out_t[:])

        tp.release()
        gp.release()
        cp.release()
        sp.release()
        pp.release()
        st_p.release()
        wp.release()

    nc.compile()
    return nc


def kernel(**inputs):
    global _BUILT
    if _BUILT is None:
        _BUILT = _build_program()
    nc = _BUILT

    f32 = np.float32
    x = np.asarray(inputs["x_t"], f32)
    hp = np.asarray(inputs["h_prev"], f32)
    slots = np.asarray(inputs["slots"], f32)
    cum = np.asarray(inputs["cum_feats"], f32)
    dt = np.asarray(inputs["delta_t"], f32)
    Wk = np.asarray(inputs["Wk"], f32)
    Wq = np.asarray(inputs["Wq"], f32)
    Wv = np.asarray(inputs["Wv"], f32)
    bv = np.asarray(inputs["bv"], f32)
    Wr = np.asarray(inputs["Wr"], f32)
    br = np.asarray(inputs["br"], f32)
    Wa = np.asarray(inputs["Wa"], f32)
    ba = np.asarray(inputs["ba"], f32)
    W_ih = np.asarray(inputs["W_ih"], f32)
    W_hh = np.asarray(inputs["W_hh"], f32)
    b_ih = np.asarray(inputs["b_ih"], f32)
    b_hh = np.asarray(inputs["b_hh"], f32)
    Wo = np.asarray(inputs["Wo"], f32)
    bo = np.asarray(inputs["bo"], f32)

    xT = np.ascontiguousarray(x.T)
    x16T = xT.astype(np.float16)
    x216T = (2.0 * xT).astype(np.float16)
    hpT = hp.T.astype(np.float16)
    s0T = slots[:, 0, :].T.astype(np.float16)
    slotsT = slots[:, 1:, :].transpose(1, 2, 0).astype(NP8)
    cumT = cum[:, 1:, :].transpose(1, 2, 0).astype(NP8)
    # delta rows per step ((delta_s+1) for s<15, zeros at s=15)
    d1 = np.zeros((S, B), np.float32)
    d1[:S - 1] = (dt[:, 1:] + 1.0).T
    d1T = d1.astype(np.float16)

    m2T = np.ascontiguousarray(Wq.T @ Wk)
    wvT = np.ascontiguousarray(Wv.T)
    wrT = np.ascontiguousarray(Wr.T)
    waT = np.ascontiguousarray(Wa.T)
    aT = np.ascontiguousarray(W_ih[:, :D].T * SC)
    cT = np.ascontiguousarray(W_ih[:, D:2 * D].T * SC)
    # rank-2 stationary: partition 0 = [bias_row; d_row], rest zero
    bd8 = np.zeros((128, 2, G4), np.float32)
    bd8[0, 0] = (b_ih + b_hh) * SC
    bd8[0, 1] = W_ih[:, 2 * D] * SC
    bd8 = bd8.astype(NP8)
    whT = (W_hh.T * SC).astype(NP8)
    woT = Wo.T.astype(np.float16)

    shared_w = {
        "m2T": m2T, "wvT": wvT, "wrT": wrT, "waT": waT,
        "bv": bv.reshape(D, 1), "br": br.reshape(1, 1), "ba": ba.reshape(1, 1),
        "bo": bo.reshape(H, 1), "aT": aT.astype(NP8), "cT": cT.astype(NP8),
        "aT16": aT.astype(np.float16), "cT16": cT.astype(np.float16),
        "bd8": bd8.reshape(128, 2 * G4), "whT": whT, "woT": woT,
    }
    in_maps = []
    for c in range(NCORES):
        lo, hi = c * BC, (c + 1) * BC
        m = dict(shared_w)
        m["xT"] = xT[:, lo:hi]
        m["x16T"] = x16T[:, lo:hi]
        m["x216T"] = x216T[:, lo:hi]
        m["hpT"] = hpT[:, lo:hi]
        m["s0T"] = s0T[:, lo:hi]
        m["slotsT"] = slotsT[:, :, lo:hi]
        m["cumT"] = cumT[:, :, lo:hi]
        m["d1T"] = np.ascontiguousarray(d1T[:, lo:hi]).reshape(1, S * BC)
        in_maps.append(m)

    res = bass_utils.run_bass_kernel_spmd(nc, in_maps, core_ids=list(range(NCORES)),
                                          **_RUN_KWARGS)
    global _LAST_RESULTS
    _LAST_RESULTS = res

    out = np.empty((B, H), np.float32)
    for c in range(NCORES):
        out[c * BC:(c + 1) * BC, :] = res.results[c]["hnT"].T
    return out


_RUN_KWARGS = {}
_LAST_RESULTS = None
